# revision 2
# baseline (speedup 1.0000x reference)
"""Multi-head causal attention (B=4, S=2048, D=1024, H=16) on 8 TRN2 cores.

Sharding: data-parallel over batch (4) x tensor-parallel over heads (2 groups
of 8 heads). Core c handles batch c//2, head-group c%2. Each core computes
q/k/v projections for its 8 heads, causal flash-style attention, and a partial
output projection against its row-shard of Wp. Host sums the two partials per
batch and adds the bias terms (bp + bv @ Wp, which commute with the row-sum).

Key layout/scheduling choices (v2):
- x arrives pre-transposed and pre-cast to bf16 from the host (xt = x.T), so
  there are no on-chip PE transposes or f32->bf16 weight casts.
- Scores are computed as S^T = kT^T qT with keys on partitions; the softmax
  denominator falls out of the AV matmul via a ones-column appended to V.
- Heads are processed in (even, odd) pairs living on partitions 0-63 / 64-127.
  The two K=64 S-matmuls of a pair are emitted back-to-back so the PE array
  row-tiles them (tile_position (0,0) / (64,0)) and runs them concurrently.
- Diagonal 128-key blocks only stream the causally-live query range (q0 =
  128*(kb-4g)); exp is likewise restricted, and a single [128,128] lower-tri
  mask handles the partial block. Fully masked regions are never written/read.
- exp/AV emission lags S emission by one unit (shared queue) so the scalar
  engine's exp and the tensor engine's S-matmuls pipeline; qkv/proj work for
  other groups is woven into the exp-bound attention stretches as fill.
"""

import numpy as np

B, S, D, H = 4, 2048, 1024, 16
HD = D // H            # head_size = 64
HPC = 8                # heads per core
LCOL = HPC * HD        # 512 local columns
NSG = 4                # seq groups of 512
SG = S // NSG          # 512
NKB = S // 128         # 16 key blocks of 128

_CACHE = {}


def _build(debug_dump=False):
    import concourse.bass as bass
    import concourse.tile as tile
    from concourse import bacc, mybir

    f32 = mybir.dt.float32
    bf16 = mybir.dt.bfloat16

    nc = bacc.Bacc("TRN2", target_bir_lowering=False, debug=False)

    xt_d = nc.dram_tensor("xt", [D, S], bf16, kind="ExternalInput")
    wq_d = nc.dram_tensor("wq", [D, LCOL], bf16, kind="ExternalInput")
    wk_d = nc.dram_tensor("wk", [D, LCOL], bf16, kind="ExternalInput")
    wv_d = nc.dram_tensor("wv", [D, LCOL], bf16, kind="ExternalInput")
    wp_d = nc.dram_tensor("wp", [LCOL, D], bf16, kind="ExternalInput")
    bq_d = nc.dram_tensor("bq", [LCOL], f32, kind="ExternalInput")
    bk_d = nc.dram_tensor("bk", [LCOL], f32, kind="ExternalInput")
    tri_d = nc.dram_tensor("tri", [128, 128], bf16, kind="ExternalInput")
    out_d = nc.dram_tensor("out", [S, D], f32, kind="ExternalOutput")

    Exp = mybir.ActivationFunctionType.Exp

    with tile.TileContext(nc) as tc:
        with (
            tc.tile_pool(name="consts", bufs=1) as consts,
            tc.tile_pool(name="xtp", bufs=2) as xtp,
            tc.tile_pool(name="acts", bufs=1) as acts,
            tc.tile_pool(name="pp", bufs=4) as pp,
            tc.tile_pool(name="recp", bufs=2) as recp,
            tc.tile_pool(name="orp", bufs=3) as orp,
            tc.tile_pool(name="rp", bufs=2) as rp,
            tc.tile_pool(name="outp", bufs=4) as outp,
            tc.tile_pool(name="drp", bufs=2, space="DRAM") as drp,
            tc.tile_pool(name="ps_s", bufs=2, space="PSUM") as ps_s,
            tc.tile_pool(name="ps_o", bufs=2, space="PSUM") as ps_o,
            tc.tile_pool(name="ps_f", bufs=2, space="PSUM") as ps_f,
        ):
            # ---- constants / weights (host already bf16, DMA direct) ------
            wq_sb = consts.tile([128, 8, LCOL], bf16)
            nc.sync.dma_start(out=wq_sb,
                              in_=wq_d.ap().rearrange("(c p) n -> p c n", p=128))
            wk_sb = consts.tile([128, 8, LCOL], bf16)
            nc.sync.dma_start(out=wk_sb,
                              in_=wk_d.ap().rearrange("(c p) n -> p c n", p=128))
            wv_sb = consts.tile([128, 8, LCOL], bf16)
            nc.sync.dma_start(out=wv_sb,
                              in_=wv_d.ap().rearrange("(c p) n -> p c n", p=128))
            wp_sb = consts.tile([128, 4, D], bf16)
            nc.sync.dma_start(out=wp_sb,
                              in_=wp_d.ap().rearrange("(c p) n -> p c n", p=128))
            tri_sb = consts.tile([128, 128], bf16)
            nc.sync.dma_start(out=tri_sb, in_=tri_d.ap())
            bq_sb = consts.tile([128, 4], f32)
            nc.sync.dma_start(out=bq_sb,
                              in_=bq_d.ap().rearrange("(c p) -> p c", p=128))
            bk_sb = consts.tile([128, 4], f32)
            nc.sync.dma_start(out=bk_sb,
                              in_=bk_d.ap().rearrange("(c p) -> p c", p=128))

            # ---- persistent activations ----------------------------------
            qT = acts.tile([128, 4, S], bf16)      # [head-dim%128, pair, seq]
            kT = acts.tile([128, 4, S], bf16)
            oT = acts.tile([128, 4, S], bf16)
            v_ext = acts.tile([128, NKB, HPC, HD + 1], bf16)
            for h in range(HPC):                   # ones columns (denominator)
                nc.vector.memset(v_ext[:, :, h, HD:HD + 1], 1.0)

            def qkv_gen(g):
                """q/k/v projections for seq group g. Chunk order: q/k for
                pair 0, then v (all kbs), then q/k for pairs 1-3 so the
                attention of (g, pair 0) can start as early as possible."""
                xT = xtp.tile([128, 8, SG], bf16, name="xT", tag="xT")
                nc.sync.dma_start(
                    out=xT,
                    in_=xt_d.ap()[:, g * SG:(g + 1) * SG]
                    .rearrange("(c p) s -> p c s", p=128))
                yield

                def qk_m(m):
                    for w_sb, b_sb, dstT in ((wq_sb, bq_sb, qT),
                                             (wk_sb, bk_sb, kT)):
                        pq = ps_f.tile([128, SG], f32, name="pq", tag="ps_f")
                        for dc in range(8):
                            nc.tensor.matmul(
                                pq, lhsT=w_sb[:, dc, 128 * m:128 * (m + 1)],
                                rhs=xT[:, dc, :], start=(dc == 0),
                                stop=(dc == 7))
                        nc.vector.tensor_scalar_add(
                            dstT[:, m, g * SG:(g + 1) * SG], pq,
                            b_sb[:, m:m + 1])
                        yield

                yield from qk_m(0)
                for s4 in range(4):
                    pv = ps_f.tile([128, LCOL], f32, name="pv", tag="ps_f")
                    for dc in range(8):
                        nc.tensor.matmul(
                            pv, lhsT=xT[:, dc, 128 * s4:128 * (s4 + 1)],
                            rhs=wv_sb[:, dc, :], start=(dc == 0), stop=(dc == 7))
                    kb = 4 * g + s4
                    nc.vector.tensor_copy(
                        v_ext[:, kb, :, 0:HD],
                        pv.rearrange("p (h e) -> p h e", e=HD))
                    yield
                for m in range(1, 4):
                    yield from qk_m(m)

            def proj_gen(g):
                for s4 in range(4):
                    sb = 4 * g + s4
                    for j in range(2):
                        ppr = ps_f.tile([128, SG], f32, name="ppr", tag="ps_f")
                        for c in range(4):
                            nc.tensor.matmul(
                                ppr, lhsT=oT[:, c, 128 * sb:128 * (sb + 1)],
                                rhs=wp_sb[:, c, j * SG:(j + 1) * SG],
                                start=(c == 0), stop=(c == 3))
                        o_sb = outp.tile([128, SG], f32, name="o_sb", tag="o_sb")
                        nc.vector.tensor_copy(o_sb, ppr)
                        nc.sync.dma_start(
                            out=out_d.ap()[128 * sb:128 * (sb + 1),
                                           j * SG:(j + 1) * SG],
                            in_=o_sb)
                        yield

            def attn_unit(ct, g, pi, pair_state, queue, tick):
                """One (head-pair, q-group, kb-pair) unit: emits the four
                row-tiled S matmuls now; queues exp+mask+AV for later."""
                nkb = 4 * g + 4
                q_sl = slice(g * SG, (g + 1) * SG)
                diag = pi >= 2 * g
                pse = ps_s.tile([128, 2, SG], f32, name="pse", tag="ps_s")
                pso = ps_s.tile([128, 2, SG], f32, name="pso", tag="ps_s")
                q0s = []
                for j in range(2):
                    kb = 2 * pi + j
                    q0 = max(0, 128 * kb - g * SG)
                    q0s.append(q0)
                    for parity, pst in ((0, pse), (1, pso)):
                        po = slice(64 * parity, 64 * parity + 64)
                        nc.tensor.matmul(
                            pst[:, j, q0:SG],
                            lhsT=kT[po, ct, 128 * kb:128 * (kb + 1)],
                            rhs=qT[po, ct, g * SG + q0:(g + 1) * SG],
                            start=True, stop=True)
                    tick()

                def exp_av():
                    for parity, pst in ((0, pse), (1, pso)):
                        h = 2 * ct + parity
                        if pi == 0:
                            pair_state[parity] = ps_o.tile(
                                [HD + 1, SG], f32, name="psum_o", tag="ps_o")
                        psum_o = pair_state[parity]
                        p_sb = pp.tile([128, 2, SG], bf16, name="p_sb",
                                       tag="p_sb")
                        if diag:
                            for j in range(2):
                                nc.scalar.activation(
                                    p_sb[:, j, q0s[j]:SG], pst[:, j, q0s[j]:SG],
                                    Exp, scale=0.125)
                            for j in range(2):
                                sl = slice(q0s[j], q0s[j] + 128)
                                nc.vector.tensor_mul(
                                    p_sb[:, j, sl], p_sb[:, j, sl], tri_sb)
                        else:
                            nc.scalar.activation(p_sb, pst, Exp, scale=0.125)
                        for j in range(2):
                            kb = 2 * pi + j
                            nc.tensor.matmul(
                                psum_o[:, q0s[j]:SG],
                                lhsT=v_ext[:, kb, h, :],
                                rhs=p_sb[:, j, q0s[j]:SG],
                                start=(kb == 0), stop=(kb == nkb - 1))

                queue.append(exp_av)
                while len(queue) > 1:
                    queue.pop(0)()

            def make_normalize(ct, g, pair_state, parity):
                def normalize():
                    po_sl = slice(64 * parity, 64 * parity + 64)
                    q_sl = slice(g * SG, (g + 1) * SG)
                    psum_o = pair_state[parity]
                    # Stage AV to SBUF right away so the PSUM bank frees fast.
                    o_raw = orp.tile([HD + 1, SG], f32, name="o_raw",
                                     tag="o_raw")
                    nc.vector.tensor_copy(o_raw, psum_o)
                    # Round-trip denominators through DRAM to spread them over
                    # 128 lanes (fast reciprocal), broadcast back via a
                    # partition-step-0 DRAM read. DMA latency only.
                    d1 = drp.tile([1, SG], f32, name="d1", tag="d1")
                    nc.sync.dma_start(out=d1, in_=o_raw[HD:HD + 1, :])
                    den_t = recp.tile([128, SG // 128], f32, name="den_t",
                                      tag="den_t")
                    nc.sync.dma_start(
                        out=den_t,
                        in_=d1.rearrange("a (c p) -> (a p) c", p=128))
                    rec_t = recp.tile([128, SG // 128], f32, name="rec_t",
                                      tag="rec_t")
                    nc.vector.reciprocal(rec_t, den_t)
                    d2 = drp.tile([1, SG], f32, name="d2", tag="d2")
                    nc.sync.dma_start(
                        out=d2.rearrange("a (c p) -> (a p) c", p=128),
                        in_=rec_t)
                    r_sb = rp.tile([HD, SG], f32, name="r_sb", tag="r_sb")
                    nc.sync.dma_start(
                        out=r_sb,
                        in_=bass.AP(tensor=d2.tensor, offset=d2.offset,
                                    ap=[[0, HD]] + [list(p) for p in d2.ap[1:]]))
                    nc.vector.tensor_mul(oT[po_sl, ct, q_sl], o_raw[0:HD, :],
                                         r_sb)
                return normalize

            # ---- schedule -------------------------------------------------
            # Prologue: xT(0) DMA + q/k for pair 0 + v for kbs 0-3, dense.
            qkv0 = qkv_gen(0)
            for _ in range(7):
                next(qkv0)

            for g in range(NSG):
                fill = []
                if g == 0:
                    fill.append(qkv0)        # remaining q/k pairs 1-3
                if g < NSG - 1:
                    fill.append(qkv_gen(g + 1))
                if g == 2:
                    fill.append(proj_gen(0))
                if g == 3:
                    fill.append(proj_gen(1))
                    fill.append(proj_gen(2))
                n_chunks = {0: 19, 1: 13, 2: 21, 3: 16}[g]
                n_ticks = 16 * (g + 1)
                stride = max(1, n_ticks // n_chunks)
                state = {"i": 0}

                def tick():
                    state["i"] += 1
                    if state["i"] % stride == 0 and fill:
                        try:
                            next(fill[0])
                        except StopIteration:
                            fill.pop(0)

                queue = []
                for ct in range(4):
                    pair_state = {}
                    for pi in range(2 * g + 2):
                        attn_unit(ct, g, pi, pair_state, queue, tick)
                    queue.append(make_normalize(ct, g, pair_state, 0))
                    queue.append(make_normalize(ct, g, pair_state, 1))
                while queue:     # group boundary: drain exp/AV + normalizes
                    queue.pop(0)()
                for gen in fill:  # drain any remaining fill chunks
                    for _ in gen:
                        pass
            for _ in proj_gen(NSG - 1):
                pass

            if debug_dump:
                for nm, t in (("qT", qT), ("kT", kT), ("v_ext", v_ext),
                              ("oT", oT)):
                    dmp = nc.dram_tensor(f"dump_{nm}", list(t.shape), bf16,
                                         kind="ExternalOutput")
                    nc.sync.dma_start(out=dmp.ap(), in_=t)

    nc.compile()
    return nc


def _get_nc():
    if "nc" not in _CACHE:
        _CACHE["nc"] = _build()
    return _CACHE["nc"]


def _make_tri():
    """tri[kl, c] = 1.0 iff kl <= c (bf16), for 128-aligned diagonal blocks."""
    import ml_dtypes
    kl = np.arange(128)[:, None]
    c = np.arange(128)[None, :]
    return (kl <= c).astype(ml_dtypes.bfloat16)


def make_in_maps(x, Wq, bq, Wk, bk, Wv, Wp):
    import ml_dtypes
    bf = ml_dtypes.bfloat16
    tri = _make_tri()
    xt = {}
    wmaps = {}
    for hg in range(2):
        hs = slice(hg * HPC, (hg + 1) * HPC)
        wmaps[hg] = {
            "wq": np.ascontiguousarray(
                Wq[hs].transpose(1, 0, 2).reshape(D, LCOL).astype(bf)),
            "wk": np.ascontiguousarray(
                Wk[hs].transpose(1, 0, 2).reshape(D, LCOL).astype(bf)),
            "wv": np.ascontiguousarray(
                Wv[hs].transpose(1, 0, 2).reshape(D, LCOL).astype(bf)),
            "wp": np.ascontiguousarray(
                Wp[hg * LCOL:(hg + 1) * LCOL, :].astype(bf)),
            "bq": np.ascontiguousarray(bq[hs].reshape(LCOL)).astype(np.float32),
            "bk": np.ascontiguousarray(bk[hs].reshape(LCOL)).astype(np.float32),
        }
    in_maps = []
    for c in range(8):
        b, hg = c // 2, c % 2
        if b not in xt:
            xt[b] = np.ascontiguousarray(np.asarray(x[b]).T.astype(bf))
        in_maps.append({"xt": xt[b], "tri": tri, **wmaps[hg]})
    return in_maps


def combine(results, Wp, bv, bp):
    """Unshard: sum the two head-group partials per batch + linear bias terms."""
    add = bp + bv.reshape(D) @ Wp
    out = np.empty((B, S, D), np.float32)
    for b in range(B):
        out[b] = results[2 * b]["out"] + results[2 * b + 1]["out"] + add
    return out


def kernel(x, Wq, bq, Wk, bk, Wv, bv, Wp, bp):
    from concourse.bass_utils import run_bass_kernel_spmd

    x = np.asarray(x, np.float32)
    Wq = np.asarray(Wq, np.float32)
    Wk = np.asarray(Wk, np.float32)
    Wv = np.asarray(Wv, np.float32)
    bq = np.asarray(bq, np.float32)
    bk = np.asarray(bk, np.float32)
    bv = np.asarray(bv, np.float32)
    Wp = np.asarray(Wp, np.float32)
    bp = np.asarray(bp, np.float32)

    nc = _get_nc()
    in_maps = make_in_maps(x, Wq, bq, Wk, bk, Wv, Wp)
    res = run_bass_kernel_spmd(nc, in_maps, core_ids=list(range(8)))
    return combine(res.results, Wp, bv, bp)


# revision 4
# speedup vs baseline: 1.3819x; 1.3819x over previous
"""Multi-head causal attention (B=4, S=2048, D=1024, H=16) on 8 TRN2 cores.

Sharding: data-parallel over batch (4) x tensor-parallel over heads (2 groups
of 8 heads). Core c handles batch c//2, head-group c%2. Each core computes
q/k/v projections for its 8 heads, causal flash-style attention, and a partial
output projection against its row-shard of Wp. Host sums the two partials per
batch and adds the bias terms (bp + bv @ Wp, which commute with the row-sum).

Key layout/scheduling choices (v3):
- x arrives pre-transposed, pre-tiled and pre-cast to bf16 from the host, as do
  all weights: DMA descriptors are 4-8KB contiguous runs (descriptor-rate is
  the startup bottleneck), and there are no on-chip transposes or casts.
- Scores are computed as S^T = kT^T qT with keys on partitions; the softmax
  denominator falls out of the AV matmul via a ones-column appended to V.
- Heads are processed in (even, odd) pairs living on partitions 0-63 / 64-127.
  The two K=64 S-matmuls of a pair are emitted back-to-back so the PE array
  row-tiles them (tile_position (0,0) / (64,0)) and runs them concurrently.
  One unit = (pair, key-block): S psum is [128, 2(parity), 512] so a single
  exp ACT covers both heads of the pair (amortizes the ~290ns ACT overhead).
- Diagonal 128-key blocks only stream the causally-live query range (q0 =
  128*(kb-4g)); exp is likewise restricted, and one [128, 2, 128] lower-tri
  mask handles the partial block. Fully masked regions are never written/read.
- exp/AV emission lags S emission by one unit (shared queue) so the scalar
  engine's exp pipelines with the tensor engine; qkv/proj work for other
  groups is woven into the exp-bound attention stretches as fill.
- Softmax denominators round-trip through DRAM for a 128-lane reciprocal; the
  final oT multiply is deferred by one head-pair so the DMA latency never
  blocks the (strict-FIFO) vector engine queue.
"""

import numpy as np

B, S, D, H = 4, 2048, 1024, 16
HD = D // H            # head_size = 64
HPC = 8                # heads per core
LCOL = HPC * HD        # 512 local columns
NSG = 4                # seq groups of 512
SG = S // NSG          # 512
NKB = S // 128         # 16 key blocks of 128

_CACHE = {}


def _build(debug_dump=False):
    import concourse.bass as bass
    import concourse.tile as tile
    from concourse import bacc, mybir

    f32 = mybir.dt.float32
    bf16 = mybir.dt.bfloat16

    nc = bacc.Bacc("TRN2", target_bir_lowering=False, debug=False)

    xtg_d = nc.dram_tensor("xtg", [NSG, 128, 8, SG], bf16, kind="ExternalInput")
    wq_d = nc.dram_tensor("wq", [128, 8, LCOL], bf16, kind="ExternalInput")
    wk_d = nc.dram_tensor("wk", [128, 8, LCOL], bf16, kind="ExternalInput")
    wv_d = nc.dram_tensor("wv", [128, 8, LCOL], bf16, kind="ExternalInput")
    wp_d = nc.dram_tensor("wp", [128, 4, D], bf16, kind="ExternalInput")
    bq_d = nc.dram_tensor("bq", [LCOL], f32, kind="ExternalInput")
    bk_d = nc.dram_tensor("bk", [LCOL], f32, kind="ExternalInput")
    tri_d = nc.dram_tensor("tri", [128, 2, 128], bf16, kind="ExternalInput")
    out_d = nc.dram_tensor("out", [S, D], f32, kind="ExternalOutput")

    Exp = mybir.ActivationFunctionType.Exp

    with tile.TileContext(nc) as tc:
        with (
            tc.tile_pool(name="consts", bufs=1) as consts,
            tc.tile_pool(name="xtp", bufs=2) as xtp,
            tc.tile_pool(name="acts", bufs=1) as acts,
            tc.tile_pool(name="pp", bufs=4) as pp,
            tc.tile_pool(name="recp", bufs=2) as recp,
            tc.tile_pool(name="orp", bufs=4) as orp,
            tc.tile_pool(name="rp", bufs=4) as rp,
            tc.tile_pool(name="outp", bufs=3) as outp,
            tc.tile_pool(name="drp", bufs=2, space="DRAM") as drp,
            tc.tile_pool(name="ps_s", bufs=2, space="PSUM") as ps_s,
            tc.tile_pool(name="ps_o", bufs=2, space="PSUM") as ps_o,
            tc.tile_pool(name="ps_f", bufs=2, space="PSUM") as ps_f,
        ):
            # ---- weights (host already bf16 + pre-tiled; 2 DMAs each) -----
            def load_w(dram, shape, name):
                t = consts.tile(shape, bf16, name=name)
                half = shape[1] // 2
                nc.sync.dma_start(out=t[:, 0:half, :], in_=dram.ap()[:, 0:half, :])
                nc.sync.dma_start(out=t[:, half:, :], in_=dram.ap()[:, half:, :])
                return t

            wq_sb = load_w(wq_d, [128, 8, LCOL], "wq_sb")
            wk_sb = load_w(wk_d, [128, 8, LCOL], "wk_sb")
            wv_sb = load_w(wv_d, [128, 8, LCOL], "wv_sb")
            wp_sb = load_w(wp_d, [128, 4, D], "wp_sb")
            tri_sb = consts.tile([128, 2, 128], bf16)
            nc.sync.dma_start(out=tri_sb, in_=tri_d.ap())
            bq_sb = consts.tile([128, 4], f32)
            nc.sync.dma_start(out=bq_sb,
                              in_=bq_d.ap().rearrange("(c p) -> p c", p=128))
            bk_sb = consts.tile([128, 4], f32)
            nc.sync.dma_start(out=bk_sb,
                              in_=bk_d.ap().rearrange("(c p) -> p c", p=128))

            # ---- persistent activations ----------------------------------
            qT = acts.tile([128, 4, S], bf16)      # [head-dim%128, pair, seq]
            kT = acts.tile([128, 4, S], bf16)
            oT = acts.tile([128, 4, S], bf16)
            v_ext = acts.tile([128, NKB, HPC, HD + 1], bf16)
            for h in range(HPC):                   # ones columns (denominator)
                nc.vector.memset(v_ext[:, :, h, HD:HD + 1], 1.0)

            def qkv_gen(g):
                """q/k/v projections for seq group g. Chunk order: q/k for
                pair 0, then v (all kbs), then q/k for pairs 1-3 so the
                attention of (g, pair 0) can start as early as possible."""
                xT = xtp.tile([128, 8, SG], bf16, name="xT", tag="xT")
                nc.sync.dma_start(out=xT[:, 0:4, :], in_=xtg_d.ap()[g][:, 0:4, :])
                nc.sync.dma_start(out=xT[:, 4:8, :], in_=xtg_d.ap()[g][:, 4:8, :])
                yield

                def qk_m(m):
                    for w_sb, b_sb, dstT in ((wq_sb, bq_sb, qT),
                                             (wk_sb, bk_sb, kT)):
                        pq = ps_f.tile([128, SG], f32, name="pq", tag="ps_f")
                        for dc in range(8):
                            nc.tensor.matmul(
                                pq, lhsT=w_sb[:, dc, 128 * m:128 * (m + 1)],
                                rhs=xT[:, dc, :], start=(dc == 0),
                                stop=(dc == 7))
                        nc.vector.tensor_scalar_add(
                            dstT[:, m, g * SG:(g + 1) * SG], pq,
                            b_sb[:, m:m + 1])
                        yield

                yield from qk_m(0)
                for s4 in range(4):
                    pv = ps_f.tile([128, LCOL], f32, name="pv", tag="ps_f")
                    for dc in range(8):
                        nc.tensor.matmul(
                            pv, lhsT=xT[:, dc, 128 * s4:128 * (s4 + 1)],
                            rhs=wv_sb[:, dc, :], start=(dc == 0), stop=(dc == 7))
                    kb = 4 * g + s4
                    nc.vector.tensor_copy(
                        v_ext[:, kb, :, 0:HD],
                        pv.rearrange("p (h e) -> p h e", e=HD))
                    yield
                for m in range(1, 4):
                    yield from qk_m(m)

            def proj_gen(g):
                for s4 in range(4):
                    sb = 4 * g + s4
                    o_sb = outp.tile([128, 2, SG], f32, name="o_sb", tag="o_sb")
                    for j in range(2):
                        ppr = ps_f.tile([128, SG], f32, name="ppr", tag="ps_f")
                        for c in range(4):
                            nc.tensor.matmul(
                                ppr, lhsT=oT[:, c, 128 * sb:128 * (sb + 1)],
                                rhs=wp_sb[:, c, j * SG:(j + 1) * SG],
                                start=(c == 0), stop=(c == 3))
                        nc.vector.tensor_copy(o_sb[:, j, :], ppr)
                        yield
                    nc.sync.dma_start(
                        out=out_d.ap()[128 * sb:128 * (sb + 1), :]
                        .rearrange("p (j n) -> p j n", j=2),
                        in_=o_sb)

            def attn_unit(ct, g, kb, pair_state, queue, tick):
                """One (head-pair, key-block) unit: emits the two row-tiled
                S matmuls now; queues exp+mask+AV for later."""
                nkb = 4 * g + 4
                q0 = max(0, 128 * kb - g * SG)
                pst = ps_s.tile([128, 2, SG], f32, name="pst", tag="ps_s")
                for parity in (0, 1):
                    po = slice(64 * parity, 64 * parity + 64)
                    nc.tensor.matmul(
                        pst[:, parity, q0:SG],
                        lhsT=kT[po, ct, 128 * kb:128 * (kb + 1)],
                        rhs=qT[po, ct, g * SG + q0:(g + 1) * SG],
                        start=True, stop=True)
                tick()

                def exp_av():
                    if kb == 0:
                        pair_state[0] = ps_o.tile([HD + 1, SG], f32,
                                                  name="po0", tag="ps_o")
                        pair_state[1] = ps_o.tile([HD + 1, SG], f32,
                                                  name="po1", tag="ps_o")
                    p_sb = pp.tile([128, 2, SG], bf16, name="p_sb", tag="p_sb")
                    nc.scalar.activation(p_sb[:, :, q0:SG], pst[:, :, q0:SG],
                                         Exp, scale=0.125)
                    if kb >= 4 * g:    # diagonal block: lower-tri mask
                        nc.vector.tensor_mul(
                            p_sb[:, :, q0:q0 + 128], p_sb[:, :, q0:q0 + 128],
                            tri_sb)
                    for parity in (0, 1):
                        h = 2 * ct + parity
                        nc.tensor.matmul(
                            pair_state[parity][:, q0:SG],
                            lhsT=v_ext[:, kb, h, :],
                            rhs=p_sb[:, parity, q0:SG],
                            start=(kb == 0), stop=(kb == nkb - 1))

                queue.append(exp_av)
                while len(queue) > 1:
                    queue.pop(0)()

            def make_norm1(ct, g, pair_state, parity, handoff):
                def norm1():
                    psum_o = pair_state[parity]
                    # Stage AV to SBUF right away so the PSUM bank frees fast.
                    o_raw = orp.tile([HD + 1, SG], f32, name="o_raw",
                                     tag="o_raw")
                    nc.vector.tensor_copy(o_raw, psum_o)
                    # Round-trip denominators through DRAM to spread them over
                    # 128 lanes (fast reciprocal), broadcast back via a
                    # partition-step-0 DRAM read. DMA latency only.
                    d1 = drp.tile([1, SG], f32, name="d1", tag="d1")
                    nc.sync.dma_start(out=d1, in_=o_raw[HD:HD + 1, :])
                    den_t = recp.tile([128, SG // 128], f32, name="den_t",
                                      tag="den_t")
                    nc.sync.dma_start(
                        out=den_t,
                        in_=d1.rearrange("a (c p) -> (a p) c", p=128))
                    rec_t = recp.tile([128, SG // 128], f32, name="rec_t",
                                      tag="rec_t")
                    nc.vector.reciprocal(rec_t, den_t)
                    d2 = drp.tile([1, SG], f32, name="d2", tag="d2")
                    nc.sync.dma_start(
                        out=d2.rearrange("a (c p) -> (a p) c", p=128),
                        in_=rec_t)
                    r_sb = rp.tile([HD, SG], f32, name="r_sb", tag="r_sb")
                    nc.sync.dma_start(
                        out=r_sb,
                        in_=bass.AP(tensor=d2.tensor, offset=d2.offset,
                                    ap=[[0, HD]] + [list(p) for p in d2.ap[1:]]))
                    handoff[parity] = (o_raw, r_sb)
                return norm1

            def make_norm2(ct, g, handoff, parity):
                def norm2():
                    po_sl = slice(64 * parity, 64 * parity + 64)
                    q_sl = slice(g * SG, (g + 1) * SG)
                    o_raw, r_sb = handoff[parity]
                    nc.vector.tensor_mul(oT[po_sl, ct, q_sl], o_raw[0:HD, :],
                                         r_sb)
                return norm2

            # ---- schedule -------------------------------------------------
            # Prologue: xT(0) DMA + q/k for pair 0 + v for kbs 0-3, dense.
            qkv0 = qkv_gen(0)
            for _ in range(7):
                next(qkv0)

            deferred = []
            for g in range(NSG):
                fill = []
                if g == 0:
                    fill.append(qkv0)        # remaining q/k pairs 1-3
                if g < NSG - 1:
                    fill.append(qkv_gen(g + 1))
                if g == 2:
                    fill.append(proj_gen(0))
                if g == 3:
                    fill.append(proj_gen(1))
                    fill.append(proj_gen(2))
                n_chunks = {0: 19, 1: 13, 2: 21, 3: 16}[g]
                n_ticks = 16 * (g + 1)
                stride = max(1, n_ticks // n_chunks)
                state = {"i": 0}

                def tick():
                    state["i"] += 1
                    if state["i"] % stride == 0 and fill:
                        try:
                            next(fill[0])
                        except StopIteration:
                            fill.pop(0)

                queue = []
                for ct in range(4):
                    pair_state = {}
                    handoff = {}
                    for kb in range(4 * g + 4):
                        attn_unit(ct, g, kb, pair_state, queue, tick)
                    queue.append(make_norm1(ct, g, pair_state, 0, handoff))
                    queue.append(make_norm1(ct, g, pair_state, 1, handoff))
                    # oT multiplies of the PREVIOUS pair: their reciprocal
                    # DMA chains have had a full pair to complete, so these
                    # never block the DVE queue head.
                    queue.extend(deferred)
                    deferred = [make_norm2(ct, g, handoff, 0),
                                make_norm2(ct, g, handoff, 1)]
                while queue:     # group boundary: drain exp/AV + norm chains
                    queue.pop(0)()
                for gen in fill:  # drain any remaining fill chunks
                    for _ in gen:
                        pass
            for fn in deferred:  # last pair's oT multiplies
                fn()
            for _ in proj_gen(NSG - 1):
                pass

            if debug_dump:
                for nm, t in (("qT", qT), ("kT", kT), ("v_ext", v_ext),
                              ("oT", oT)):
                    dmp = nc.dram_tensor(f"dump_{nm}", list(t.shape), bf16,
                                         kind="ExternalOutput")
                    nc.sync.dma_start(out=dmp.ap(), in_=t)

    nc.compile()
    return nc


def _get_nc():
    if "nc" not in _CACHE:
        _CACHE["nc"] = _build()
    return _CACHE["nc"]


def _make_tri():
    """tri[kl, :, c] = 1.0 iff kl <= c (bf16), for 128-wide diagonal blocks,
    duplicated on axis 1 so one DVE multiply covers both heads of a pair."""
    import ml_dtypes
    kl = np.arange(128)[:, None]
    c = np.arange(128)[None, :]
    t = (kl <= c).astype(ml_dtypes.bfloat16)
    return np.ascontiguousarray(np.broadcast_to(t[:, None, :], (128, 2, 128)))


def make_in_maps(x, Wq, bq, Wk, bk, Wv, Wp):
    import ml_dtypes
    bf = ml_dtypes.bfloat16
    tri = _make_tri()
    xt = {}
    wmaps = {}

    def tile_w(w2d, chunks):
        # [128*chunks, n] -> [128, chunks, n] with 128c+p row mapping
        n = w2d.shape[1]
        return np.ascontiguousarray(
            w2d.reshape(chunks, 128, n).transpose(1, 0, 2).astype(bf))

    for hg in range(2):
        hs = slice(hg * HPC, (hg + 1) * HPC)
        wmaps[hg] = {
            "wq": tile_w(Wq[hs].transpose(1, 0, 2).reshape(D, LCOL), 8),
            "wk": tile_w(Wk[hs].transpose(1, 0, 2).reshape(D, LCOL), 8),
            "wv": tile_w(Wv[hs].transpose(1, 0, 2).reshape(D, LCOL), 8),
            "wp": tile_w(Wp[hg * LCOL:(hg + 1) * LCOL, :], 4),
            "bq": np.ascontiguousarray(bq[hs].reshape(LCOL)).astype(np.float32),
            "bk": np.ascontiguousarray(bk[hs].reshape(LCOL)).astype(np.float32),
        }
    in_maps = []
    for c in range(8):
        b, hg = c // 2, c % 2
        if b not in xt:
            # xtg[g, p, c, s] = x[b][512g+s, 128c+p]
            xt[b] = np.ascontiguousarray(
                np.asarray(x[b]).reshape(NSG, SG, 8, 128)
                .transpose(0, 3, 2, 1).astype(bf))
        in_maps.append({"xtg": xt[b], "tri": tri, **wmaps[hg]})
    return in_maps


def combine(results, Wp, bv, bp):
    """Unshard: sum the two head-group partials per batch + linear bias terms."""
    add = bp + bv.reshape(D) @ Wp
    out = np.empty((B, S, D), np.float32)
    for b in range(B):
        out[b] = results[2 * b]["out"] + results[2 * b + 1]["out"] + add
    return out


def kernel(x, Wq, bq, Wk, bk, Wv, bv, Wp, bp):
    from concourse.bass_utils import run_bass_kernel_spmd

    x = np.asarray(x, np.float32)
    Wq = np.asarray(Wq, np.float32)
    Wk = np.asarray(Wk, np.float32)
    Wv = np.asarray(Wv, np.float32)
    bq = np.asarray(bq, np.float32)
    bk = np.asarray(bk, np.float32)
    bv = np.asarray(bv, np.float32)
    Wp = np.asarray(Wp, np.float32)
    bp = np.asarray(bp, np.float32)

    nc = _get_nc()
    in_maps = make_in_maps(x, Wq, bq, Wk, bk, Wv, Wp)
    res = run_bass_kernel_spmd(nc, in_maps, core_ids=list(range(8)))
    return combine(res.results, Wp, bv, bp)


# revision 7
# speedup vs baseline: 2.0253x; 1.4656x over previous
"""Multi-head causal attention (B=4, S=2048, D=1024, H=16) on 8 TRN2 cores.

Sharding: data-parallel over batch (4) x tensor-parallel over heads (2 groups
of 8 heads). Core c handles batch c//2, head-group c%2. Each core computes
q/k/v projections for its 8 heads, causal flash-style attention, and a partial
output projection against its row-shard of Wp. Host sums the two partials per
batch and adds the bias terms (bp + bv @ Wp, which commute with the row-sum).

Key layout/scheduling choices (v3):
- x arrives pre-transposed, pre-tiled and pre-cast to bf16 from the host, as do
  all weights: DMA descriptors are 4-8KB contiguous runs (descriptor-rate is
  the startup bottleneck), and there are no on-chip transposes or casts.
- Scores are computed as S^T = kT^T qT with keys on partitions; the softmax
  denominator falls out of the AV matmul via a ones-column appended to V.
- Heads are processed in (even, odd) pairs living on partitions 0-63 / 64-127.
  The two K=64 S-matmuls of a pair are emitted back-to-back so the PE array
  row-tiles them (tile_position (0,0) / (64,0)) and runs them concurrently.
  One unit = (pair, key-block): S psum is [128, 2(parity), 512] so a single
  exp ACT covers both heads of the pair (amortizes the ~290ns ACT overhead).
- Diagonal 128-key blocks only stream the causally-live query range (q0 =
  128*(kb-4g)); exp is likewise restricted, and one [128, 2, 128] lower-tri
  mask handles the partial block. Fully masked regions are never written/read.
- exp/AV emission lags S emission by one unit (shared queue) so the scalar
  engine's exp pipelines with the tensor engine; qkv/proj work for other
  groups is woven into the exp-bound attention stretches as fill.
- Softmax denominators round-trip through DRAM for a 128-lane reciprocal; the
  final oT multiply is deferred by one head-pair so the DMA latency never
  blocks the (strict-FIFO) vector engine queue.
"""

import numpy as np

B, S, D, H = 4, 2048, 1024, 16
HD = D // H            # head_size = 64
HPC = 8                # heads per core
LCOL = HPC * HD        # 512 local columns
NSG = 4                # seq groups of 512
SG = S // NSG          # 512
NKB = S // 128         # 16 key blocks of 128

_CACHE = {}


def _build(debug_dump=False):
    import concourse.bass as bass
    import concourse.tile as tile
    from concourse import bacc, mybir

    f32 = mybir.dt.float32
    bf16 = mybir.dt.bfloat16

    nc = bacc.Bacc("TRN2", target_bir_lowering=False, debug=False)

    xtg_d = nc.dram_tensor("xtg", [NSG, 128, 8, SG], bf16, kind="ExternalInput")
    wq_d = nc.dram_tensor("wq", [128, 8, LCOL], bf16, kind="ExternalInput")
    wk_d = nc.dram_tensor("wk", [128, 8, LCOL], bf16, kind="ExternalInput")
    wv_d = nc.dram_tensor("wv", [128, 8, LCOL], bf16, kind="ExternalInput")
    wp_d = nc.dram_tensor("wp", [128, 4, D], bf16, kind="ExternalInput")
    bq_d = nc.dram_tensor("bq", [LCOL], f32, kind="ExternalInput")
    bk_d = nc.dram_tensor("bk", [LCOL], f32, kind="ExternalInput")
    tri_d = nc.dram_tensor("tri", [128, 2, 128], bf16, kind="ExternalInput")
    out_d = nc.dram_tensor("out", [S, D], f32, kind="ExternalOutput")

    Exp = mybir.ActivationFunctionType.Exp

    with tile.TileContext(nc) as tc:
        with (
            tc.tile_pool(name="consts", bufs=1) as consts,
            tc.tile_pool(name="xtp", bufs=2) as xtp,
            tc.tile_pool(name="acts", bufs=1) as acts,
            tc.tile_pool(name="pp", bufs=4) as pp,
            tc.tile_pool(name="recp", bufs=4) as recp,
            tc.tile_pool(name="orp", bufs=6) as orp,
            tc.tile_pool(name="rp", bufs=4) as rp,
            tc.tile_pool(name="outp", bufs=3) as outp,
            tc.tile_pool(name="drp", bufs=2, space="DRAM") as drp,
            tc.tile_pool(name="ps_s", bufs=2, space="PSUM") as ps_s,
            tc.tile_pool(name="ps_o", bufs=2, space="PSUM") as ps_o,
            tc.tile_pool(name="ps_f", bufs=2, space="PSUM") as ps_f,
        ):
            # ---- weights (host already bf16 + pre-tiled; 2 DMAs each) -----
            def load_w(dram, shape, name):
                t = consts.tile(shape, bf16, name=name)
                half = shape[1] // 2
                nc.sync.dma_start(out=t[:, 0:half, :], in_=dram.ap()[:, 0:half, :])
                nc.sync.dma_start(out=t[:, half:, :], in_=dram.ap()[:, half:, :])
                return t

            wq_sb = load_w(wq_d, [128, 8, LCOL], "wq_sb")
            wk_sb = load_w(wk_d, [128, 8, LCOL], "wk_sb")
            wv_sb = load_w(wv_d, [128, 8, LCOL], "wv_sb")
            wp_sb = load_w(wp_d, [128, 4, D], "wp_sb")
            tri_sb = consts.tile([128, 2, 128], bf16)
            nc.sync.dma_start(out=tri_sb, in_=tri_d.ap())
            bq_sb = consts.tile([128, 4], f32)
            nc.sync.dma_start(out=bq_sb,
                              in_=bq_d.ap().rearrange("(c p) -> p c", p=128))
            bk_sb = consts.tile([128, 4], f32)
            nc.sync.dma_start(out=bk_sb,
                              in_=bk_d.ap().rearrange("(c p) -> p c", p=128))

            # ---- persistent activations ----------------------------------
            qT = acts.tile([128, 4, S], bf16)      # [head-dim%128, pair, seq]
            kT = acts.tile([128, 4, S], bf16)
            oT = acts.tile([128, 4, S], bf16)
            v_ext = acts.tile([128, NKB, HPC, HD + 1], bf16)
            for h in range(HPC):                   # ones columns (denominator)
                nc.vector.memset(v_ext[:, :, h, HD:HD + 1], 1.0)

            def qkv_gen(g):
                """q/k/v projections for seq group g. Chunk order: q/k for
                pair 0, then v (all kbs), then q/k for pairs 1-3 so the
                attention of (g, pair 0) can start as early as possible."""
                xT = xtp.tile([128, 8, SG], bf16, name="xT", tag="xT")
                nc.sync.dma_start(out=xT[:, 0:4, :], in_=xtg_d.ap()[g][:, 0:4, :])
                nc.sync.dma_start(out=xT[:, 4:8, :], in_=xtg_d.ap()[g][:, 4:8, :])
                yield

                def qk_m(m):
                    for w_sb, b_sb, dstT in ((wq_sb, bq_sb, qT),
                                             (wk_sb, bk_sb, kT)):
                        pq = ps_f.tile([128, SG], f32, name="pq", tag="ps_f")
                        for dc in range(8):
                            nc.tensor.matmul(
                                pq, lhsT=w_sb[:, dc, 128 * m:128 * (m + 1)],
                                rhs=xT[:, dc, :], start=(dc == 0),
                                stop=(dc == 7))
                        nc.vector.tensor_scalar_add(
                            dstT[:, m, g * SG:(g + 1) * SG], pq,
                            b_sb[:, m:m + 1])
                        yield

                yield from qk_m(0)
                for s4 in range(4):
                    pv = ps_f.tile([128, LCOL], f32, name="pv", tag="ps_f")
                    for dc in range(8):
                        nc.tensor.matmul(
                            pv, lhsT=xT[:, dc, 128 * s4:128 * (s4 + 1)],
                            rhs=wv_sb[:, dc, :], start=(dc == 0), stop=(dc == 7))
                    kb = 4 * g + s4
                    nc.vector.tensor_copy(
                        v_ext[:, kb, :, 0:HD],
                        pv.rearrange("p (h e) -> p h e", e=HD))
                    yield
                for m in range(1, 4):
                    yield from qk_m(m)

            def proj_gen(g):
                for s4 in range(4):
                    sb = 4 * g + s4
                    o_sb = outp.tile([128, 2, SG], f32, name="o_sb", tag="o_sb")
                    for j in range(2):
                        ppr = ps_f.tile([128, SG], f32, name="ppr", tag="ps_f")
                        for c in range(4):
                            nc.tensor.matmul(
                                ppr, lhsT=oT[:, c, 128 * sb:128 * (sb + 1)],
                                rhs=wp_sb[:, c, j * SG:(j + 1) * SG],
                                start=(c == 0), stop=(c == 3))
                        nc.vector.tensor_copy(o_sb[:, j, :], ppr)
                        yield
                    nc.sync.dma_start(
                        out=out_d.ap()[128 * sb:128 * (sb + 1), :]
                        .rearrange("p (j n) -> p j n", j=2),
                        in_=o_sb)

            def attn_unit(ct, g, kb, pair_state, queue, tick):
                """One (head-pair, key-block) unit: emits the two row-tiled
                S matmuls now; queues exp+mask+AV for later."""
                nkb = 4 * g + 4
                q0 = max(0, 128 * kb - g * SG)
                pst = ps_s.tile([128, 2, SG], f32, name="pst", tag="ps_s")
                for parity in (0, 1):
                    po = slice(64 * parity, 64 * parity + 64)
                    nc.tensor.matmul(
                        pst[:, parity, q0:SG],
                        lhsT=kT[po, ct, 128 * kb:128 * (kb + 1)],
                        rhs=qT[po, ct, g * SG + q0:(g + 1) * SG],
                        start=True, stop=True)
                tick()

                def exp_av():
                    if kb == 0:
                        pair_state[0] = ps_o.tile([HD + 1, SG], f32,
                                                  name="po0", tag="ps_o")
                        pair_state[1] = ps_o.tile([HD + 1, SG], f32,
                                                  name="po1", tag="ps_o")
                    p_sb = pp.tile([128, 2, SG], bf16, name="p_sb", tag="p_sb")
                    nc.scalar.activation(p_sb[:, :, q0:SG], pst[:, :, q0:SG],
                                         Exp, scale=0.125)
                    if kb >= 4 * g:    # diagonal block: lower-tri mask
                        nc.vector.tensor_mul(
                            p_sb[:, :, q0:q0 + 128], p_sb[:, :, q0:q0 + 128],
                            tri_sb)
                    for parity in (0, 1):
                        h = 2 * ct + parity
                        nc.tensor.matmul(
                            pair_state[parity][:, q0:SG],
                            lhsT=v_ext[:, kb, h, :],
                            rhs=p_sb[:, parity, q0:SG],
                            start=(kb == 0), stop=(kb == nkb - 1))

                queue.append(exp_av)
                while len(queue) > 1:
                    queue.pop(0)()

            # Normalization runs as a 3-stage pipeline, each stage deferred
            # by one head-pair so no DVE op ever waits on a DMA round-trip
            # at the head of the (strict FIFO) vector queue.
            def make_normA(ct, g, pair_state, parity, handoff):
                def normA():
                    psum_o = pair_state[parity]
                    # Stage AV to SBUF right away so the PSUM bank frees fast.
                    o_raw = orp.tile([HD + 1, SG], f32, name="o_raw",
                                     tag="o_raw")
                    nc.vector.tensor_copy(o_raw, psum_o)
                    # Round-trip denominators through DRAM to spread them over
                    # 128 lanes (fast reciprocal), broadcast back via a
                    # partition-step-0 DRAM read. DMA latency only.
                    d1 = drp.tile([1, SG], f32, name="d1", tag="d1")
                    nc.sync.dma_start(out=d1, in_=o_raw[HD:HD + 1, :])
                    den_t = recp.tile([128, SG // 128], f32, name="den_t",
                                      tag="den_t")
                    nc.sync.dma_start(
                        out=den_t,
                        in_=d1.rearrange("a (p c) -> (a p) c", p=128))
                    handoff[parity] = [o_raw, den_t]
                return normA

            def make_normB(ct, g, handoff, parity):
                def normB():
                    o_raw, den_t = handoff[parity]
                    rec_t = recp.tile([128, SG // 128], f32, name="rec_t",
                                      tag="rec_t")
                    nc.vector.reciprocal(rec_t, den_t)
                    d2 = drp.tile([1, SG], f32, name="d2", tag="d2")
                    nc.sync.dma_start(
                        out=d2.rearrange("a (p c) -> (a p) c", p=128),
                        in_=rec_t)
                    r_sb = rp.tile([HD, SG], f32, name="r_sb", tag="r_sb")
                    nc.sync.dma_start(
                        out=r_sb,
                        in_=bass.AP(tensor=d2.tensor, offset=d2.offset,
                                    ap=[[0, HD]] + [list(p) for p in d2.ap[1:]]))
                    handoff[parity] = [o_raw, r_sb]
                return normB

            def make_normC(ct, g, handoff, parity):
                def normC():
                    po_sl = slice(64 * parity, 64 * parity + 64)
                    q_sl = slice(g * SG, (g + 1) * SG)
                    o_raw, r_sb = handoff[parity]
                    nc.vector.tensor_mul(oT[po_sl, ct, q_sl], o_raw[0:HD, :],
                                         r_sb)
                return normC

            # ---- schedule -------------------------------------------------
            # Prologue: xT(0) DMA + q/k for pair 0 + v for kbs 0-3, dense.
            qkv0 = qkv_gen(0)
            for _ in range(7):
                next(qkv0)

            defB, defC, defC_next = [], [], []
            for g in range(NSG):
                fill = []
                if g == 0:
                    fill.append(qkv0)        # remaining q/k pairs 1-3
                if g < NSG - 1:
                    fill.append(qkv_gen(g + 1))
                if g == 2:
                    fill.append(proj_gen(0))
                if g == 3:
                    fill.append(proj_gen(1))
                    fill.append(proj_gen(2))
                # g3 stride underfeeds on purpose: the leftover proj chunks
                # drain at the end, covering the final normalize latency.
                stride = {0: 1, 1: 2, 2: 2, 3: 5}[g]
                state = {"i": 0}

                def tick():
                    state["i"] += 1
                    if state["i"] % stride == 0 and fill:
                        try:
                            next(fill[0])
                        except StopIteration:
                            fill.pop(0)

                queue = []
                for ct in range(4):
                    pair_state = {}
                    handoff = {}
                    for kb in range(4 * g + 4):
                        attn_unit(ct, g, kb, pair_state, queue, tick)
                    queue.append(make_normA(ct, g, pair_state, 0, handoff))
                    queue.append(make_normA(ct, g, pair_state, 1, handoff))
                    queue.extend(defB)       # pair ct-1: reciprocal + spread
                    queue.extend(defC)       # pair ct-2: oT multiply
                    defC = defC_next
                    defC_next = [make_normC(ct, g, handoff, 0),
                                 make_normC(ct, g, handoff, 1)]
                    defB = [make_normB(ct, g, handoff, 0),
                            make_normB(ct, g, handoff, 1)]
                while queue:     # group boundary: drain exp/AV + norm chains
                    queue.pop(0)()
                for gen in fill:  # drain any remaining fill chunks
                    for _ in gen:
                        pass
            for fn in defC + defB + defC_next:   # last pairs' norm stages
                fn()
            for _ in proj_gen(NSG - 1):
                pass

            if debug_dump:
                for nm, t in (("qT", qT), ("kT", kT), ("v_ext", v_ext),
                              ("oT", oT)):
                    dmp = nc.dram_tensor(f"dump_{nm}", list(t.shape), bf16,
                                         kind="ExternalOutput")
                    nc.sync.dma_start(out=dmp.ap(), in_=t)

    nc.compile()
    return nc


def _get_nc():
    if "nc" not in _CACHE:
        _CACHE["nc"] = _build()
    return _CACHE["nc"]


def _make_tri():
    """tri[kl, :, c] = 1.0 iff kl <= c (bf16), for 128-wide diagonal blocks,
    duplicated on axis 1 so one DVE multiply covers both heads of a pair."""
    import ml_dtypes
    kl = np.arange(128)[:, None]
    c = np.arange(128)[None, :]
    t = (kl <= c).astype(ml_dtypes.bfloat16)
    return np.ascontiguousarray(np.broadcast_to(t[:, None, :], (128, 2, 128)))


def make_in_maps(x, Wq, bq, Wk, bk, Wv, Wp):
    import ml_dtypes
    bf = ml_dtypes.bfloat16
    tri = _make_tri()
    xt = {}
    wmaps = {}

    def tile_w(w2d, chunks):
        # [128*chunks, n] -> [128, chunks, n] with 128c+p row mapping
        n = w2d.shape[1]
        return np.ascontiguousarray(
            w2d.reshape(chunks, 128, n).transpose(1, 0, 2).astype(bf))

    for hg in range(2):
        hs = slice(hg * HPC, (hg + 1) * HPC)
        wmaps[hg] = {
            "wq": tile_w(Wq[hs].transpose(1, 0, 2).reshape(D, LCOL), 8),
            "wk": tile_w(Wk[hs].transpose(1, 0, 2).reshape(D, LCOL), 8),
            "wv": tile_w(Wv[hs].transpose(1, 0, 2).reshape(D, LCOL), 8),
            "wp": tile_w(Wp[hg * LCOL:(hg + 1) * LCOL, :], 4),
            "bq": np.ascontiguousarray(bq[hs].reshape(LCOL)).astype(np.float32),
            "bk": np.ascontiguousarray(bk[hs].reshape(LCOL)).astype(np.float32),
        }
    in_maps = []
    for c in range(8):
        b, hg = c // 2, c % 2
        if b not in xt:
            # xtg[g, p, c, s] = x[b][512g+s, 128c+p]
            xt[b] = np.ascontiguousarray(
                np.asarray(x[b]).reshape(NSG, SG, 8, 128)
                .transpose(0, 3, 2, 1).astype(bf))
        in_maps.append({"xtg": xt[b], "tri": tri, **wmaps[hg]})
    return in_maps


def combine(results, Wp, bv, bp):
    """Unshard: sum the two head-group partials per batch + linear bias terms."""
    add = bp + bv.reshape(D) @ Wp
    out = np.empty((B, S, D), np.float32)
    for b in range(B):
        out[b] = results[2 * b]["out"] + results[2 * b + 1]["out"] + add
    return out


def kernel(x, Wq, bq, Wk, bk, Wv, bv, Wp, bp):
    from concourse.bass_utils import run_bass_kernel_spmd

    x = np.asarray(x, np.float32)
    Wq = np.asarray(Wq, np.float32)
    Wk = np.asarray(Wk, np.float32)
    Wv = np.asarray(Wv, np.float32)
    bq = np.asarray(bq, np.float32)
    bk = np.asarray(bk, np.float32)
    bv = np.asarray(bv, np.float32)
    Wp = np.asarray(Wp, np.float32)
    bp = np.asarray(bp, np.float32)

    nc = _get_nc()
    in_maps = make_in_maps(x, Wq, bq, Wk, bk, Wv, Wp)
    res = run_bass_kernel_spmd(nc, in_maps, core_ids=list(range(8)))
    return combine(res.results, Wp, bv, bp)


# revision 11
# speedup vs baseline: 2.1844x; 1.0785x over previous
"""Multi-head causal attention (B=4, S=2048, D=1024, H=16) on 8 TRN2 cores.

Sharding: data-parallel over batch (4) x tensor-parallel over heads (2 groups
of 8 heads). Core c handles batch c//2, head-group c%2. Each core computes
q/k/v projections for its 8 heads, causal flash-style attention, and a partial
output projection against its row-shard of Wp. Host sums the two partials per
batch and adds the bias terms (bp + bv @ Wp, which commute with the row-sum).

Key layout/scheduling choices (v3):
- x arrives pre-transposed, pre-tiled and pre-cast to bf16 from the host, as do
  all weights: DMA descriptors are 4-8KB contiguous runs (descriptor-rate is
  the startup bottleneck), and there are no on-chip transposes or casts.
- Scores are computed as S^T = kT^T qT with keys on partitions; the softmax
  denominator falls out of the AV matmul via a ones-column appended to V.
- Heads are processed in (even, odd) pairs living on partitions 0-63 / 64-127.
  The two K=64 S-matmuls of a pair are emitted back-to-back so the PE array
  row-tiles them (tile_position (0,0) / (64,0)) and runs them concurrently.
  One unit = (pair, key-block): S psum is [128, 2(parity), 512] so a single
  exp ACT covers both heads of the pair (amortizes the ~290ns ACT overhead).
- Diagonal 128-key blocks only stream the causally-live query range (q0 =
  128*(kb-4g)); exp is likewise restricted, and one [128, 2, 128] lower-tri
  mask handles the partial block. Fully masked regions are never written/read.
- exp/AV emission lags S emission by one unit (shared queue) so the scalar
  engine's exp pipelines with the tensor engine; qkv/proj work for other
  groups is woven into the exp-bound attention stretches as fill.
- Softmax denominators round-trip through DRAM for a 128-lane reciprocal; the
  final oT multiply is deferred by one head-pair so the DMA latency never
  blocks the (strict-FIFO) vector engine queue.
"""

import numpy as np

B, S, D, H = 4, 2048, 1024, 16
HD = D // H            # head_size = 64
HPC = 8                # heads per core
LCOL = HPC * HD        # 512 local columns
NSG = 4                # seq groups of 512
SG = S // NSG          # 512
NKB = S // 128         # 16 key blocks of 128

_CACHE = {}


def _build(debug_dump=False):
    import concourse.bass as bass
    import concourse.tile as tile
    from concourse import bacc, mybir

    f32 = mybir.dt.float32
    bf16 = mybir.dt.bfloat16

    nc = bacc.Bacc("TRN2", target_bir_lowering=False, debug=False)

    xtg_d = nc.dram_tensor("xtg", [NSG, 128, 8, SG], bf16, kind="ExternalInput")
    wq_d = nc.dram_tensor("wq", [128, 8, LCOL], bf16, kind="ExternalInput")
    wk_d = nc.dram_tensor("wk", [128, 8, LCOL], bf16, kind="ExternalInput")
    wv_d = nc.dram_tensor("wv", [128, 8, LCOL], bf16, kind="ExternalInput")
    wp_d = nc.dram_tensor("wp", [128, 4, D], bf16, kind="ExternalInput")
    bq_d = nc.dram_tensor("bq", [LCOL], f32, kind="ExternalInput")
    bk_d = nc.dram_tensor("bk", [LCOL], f32, kind="ExternalInput")
    tri_d = nc.dram_tensor("tri", [128, 2, 128], bf16, kind="ExternalInput")
    out_d = nc.dram_tensor("out", [S, D], f32, kind="ExternalOutput")

    Exp = mybir.ActivationFunctionType.Exp

    with tile.TileContext(nc) as tc:
        with (
            tc.tile_pool(name="consts", bufs=1) as consts,
            tc.tile_pool(name="xtp", bufs=2) as xtp,
            tc.tile_pool(name="acts", bufs=1) as acts,
            tc.tile_pool(name="pp", bufs=4) as pp,
            tc.tile_pool(name="recp", bufs=4) as recp,
            tc.tile_pool(name="orp", bufs=6) as orp,
            tc.tile_pool(name="rp", bufs=4) as rp,
            tc.tile_pool(name="outp", bufs=3) as outp,
            tc.tile_pool(name="drp", bufs=2, space="DRAM") as drp,
            tc.tile_pool(name="ps_s", bufs=2, space="PSUM") as ps_s,
            tc.tile_pool(name="ps_o", bufs=2, space="PSUM") as ps_o,
            tc.tile_pool(name="ps_f", bufs=2, space="PSUM") as ps_f,
        ):
            # ---- weights (host already bf16 + pre-tiled; 2 DMAs each) -----
            def load_w(dram, shape, name):
                t = consts.tile(shape, bf16, name=name)
                half = shape[1] // 2
                nc.sync.dma_start(out=t[:, 0:half, :], in_=dram.ap()[:, 0:half, :])
                nc.sync.dma_start(out=t[:, half:, :], in_=dram.ap()[:, half:, :])
                return t



            # ---- persistent activations ----------------------------------
            qT = acts.tile([128, 4, S], bf16)      # [head-dim%128, pair, seq]
            kT = acts.tile([128, 4, S], bf16)
            oT = acts.tile([128, 4, S], bf16)
            v_ext = acts.tile([128, NKB, HPC, HD + 1], bf16)
            for h in range(HPC):                   # ones columns (denominator)
                nc.vector.memset(v_ext[:, :, h, HD:HD + 1], 1.0)

            def qkv_gen(g):
                """q/k/v projections for seq group g. Chunk order: q/k for
                pair 0, then v (all kbs), then q/k for pairs 1-3 so the
                attention of (g, pair 0) can start as early as possible."""
                xT = xtp.tile([128, 8, SG], bf16, name="xT", tag="xT")
                nc.sync.dma_start(out=xT[:, 0:4, :], in_=xtg_d.ap()[g][:, 0:4, :])
                nc.sync.dma_start(out=xT[:, 4:8, :], in_=xtg_d.ap()[g][:, 4:8, :])
                yield

                def qk_m(m):
                    for w_sb, b_sb, dstT in ((wq_sb, bq_sb, qT),
                                             (wk_sb, bk_sb, kT)):
                        pq = ps_f.tile([128, SG], f32, name="pq", tag="ps_f")
                        for dc in range(8):
                            nc.tensor.matmul(
                                pq, lhsT=w_sb[:, dc, 128 * m:128 * (m + 1)],
                                rhs=xT[:, dc, :], start=(dc == 0),
                                stop=(dc == 7))
                        nc.vector.tensor_scalar_add(
                            dstT[:, m, g * SG:(g + 1) * SG], pq,
                            b_sb[:, m:m + 1])
                        yield

                yield from qk_m(0)
                for s4 in range(4):
                    pv = ps_f.tile([128, LCOL], f32, name="pv", tag="ps_f")
                    for dc in range(8):
                        nc.tensor.matmul(
                            pv, lhsT=xT[:, dc, 128 * s4:128 * (s4 + 1)],
                            rhs=wv_sb[:, dc, :], start=(dc == 0), stop=(dc == 7))
                    kb = 4 * g + s4
                    nc.vector.tensor_copy(
                        v_ext[:, kb, :, 0:HD],
                        pv.rearrange("p (h e) -> p h e", e=HD))
                    yield
                for m in range(1, 4):
                    yield from qk_m(m)

            def proj_gen(g):
                for s4 in range(4):
                    sb = 4 * g + s4
                    o_sb = outp.tile([128, 2, SG], f32, name="o_sb", tag="o_sb")
                    for j in range(2):
                        ppr = ps_f.tile([128, SG], f32, name="ppr", tag="ps_f")
                        for c in range(4):
                            nc.tensor.matmul(
                                ppr, lhsT=oT[:, c, 128 * sb:128 * (sb + 1)],
                                rhs=wp_sb[:, c, j * SG:(j + 1) * SG],
                                start=(c == 0), stop=(c == 3))
                        nc.vector.tensor_copy(o_sb[:, j, :], ppr)
                        yield
                    nc.sync.dma_start(
                        out=out_d.ap()[128 * sb:128 * (sb + 1), :]
                        .rearrange("p (j n) -> p j n", j=2),
                        in_=o_sb)

            def attn_unit(ct, g, kb, pair_state, queue, tick):
                """One (head-pair, key-block) unit: emits the two row-tiled
                S matmuls now; queues exp+mask+AV for later."""
                nkb = 4 * g + 4
                q0 = max(0, 128 * kb - g * SG)
                pst = ps_s.tile([128, 2, SG], f32, name="pst", tag="ps_s")
                for parity in (0, 1):
                    po = slice(64 * parity, 64 * parity + 64)
                    nc.tensor.matmul(
                        pst[:, parity, q0:SG],
                        lhsT=kT[po, ct, 128 * kb:128 * (kb + 1)],
                        rhs=qT[po, ct, g * SG + q0:(g + 1) * SG],
                        start=True, stop=True)
                tick()

                def exp_av():
                    if kb == 0:
                        pair_state[0] = ps_o.tile([HD + 1, SG], f32,
                                                  name="po0", tag="ps_o")
                        pair_state[1] = ps_o.tile([HD + 1, SG], f32,
                                                  name="po1", tag="ps_o")
                    p_sb = pp.tile([128, 2, SG], bf16, name="p_sb", tag="p_sb")
                    nc.scalar.activation(p_sb[:, :, q0:SG], pst[:, :, q0:SG],
                                         Exp, scale=0.125)
                    if kb >= 4 * g:    # diagonal block: lower-tri mask
                        nc.vector.tensor_mul(
                            p_sb[:, :, q0:q0 + 128], p_sb[:, :, q0:q0 + 128],
                            tri_sb)
                    for parity in (0, 1):
                        h = 2 * ct + parity
                        nc.tensor.matmul(
                            pair_state[parity][:, q0:SG],
                            lhsT=v_ext[:, kb, h, :],
                            rhs=p_sb[:, parity, q0:SG],
                            start=(kb == 0), stop=(kb == nkb - 1))

                queue.append(exp_av)
                while len(queue) > 1:
                    queue.pop(0)()

            # Normalization runs as a 3-stage pipeline, each stage deferred
            # by one head-pair so no DVE op ever waits on a DMA round-trip
            # at the head of the (strict FIFO) vector queue.
            def make_normA(ct, g, pair_state, parity, handoff):
                def normA():
                    psum_o = pair_state[parity]
                    # Stage AV to SBUF right away so the PSUM bank frees fast.
                    o_raw = orp.tile([HD + 1, SG], f32, name="o_raw",
                                     tag="o_raw")
                    nc.vector.tensor_copy(o_raw, psum_o)
                    # Round-trip denominators through DRAM to spread them over
                    # 128 lanes (fast reciprocal), broadcast back via a
                    # partition-step-0 DRAM read. DMA latency only.
                    d1 = drp.tile([1, SG], f32, name="d1", tag="d1")
                    nc.sync.dma_start(out=d1, in_=o_raw[HD:HD + 1, :])
                    den_t = recp.tile([128, SG // 128], f32, name="den_t",
                                      tag="den_t")
                    nc.sync.dma_start(
                        out=den_t,
                        in_=d1.rearrange("a (p c) -> (a p) c", p=128))
                    handoff[parity] = [o_raw, den_t]
                return normA

            def make_normB(ct, g, handoff, parity):
                def normB():
                    o_raw, den_t = handoff[parity]
                    rec_t = recp.tile([128, SG // 128], f32, name="rec_t",
                                      tag="rec_t")
                    nc.vector.reciprocal(rec_t, den_t)
                    d2 = drp.tile([1, SG], f32, name="d2", tag="d2")
                    nc.sync.dma_start(
                        out=d2.rearrange("a (p c) -> (a p) c", p=128),
                        in_=rec_t)
                    r_sb = rp.tile([HD, SG], f32, name="r_sb", tag="r_sb")
                    nc.sync.dma_start(
                        out=r_sb,
                        in_=bass.AP(tensor=d2.tensor, offset=d2.offset,
                                    ap=[[0, HD]] + [list(p) for p in d2.ap[1:]]))
                    handoff[parity] = [o_raw, r_sb]
                return normB

            def make_normC(ct, g, handoff, parity):
                def normC():
                    po_sl = slice(64 * parity, 64 * parity + 64)
                    q_sl = slice(g * SG, (g + 1) * SG)
                    o_raw, r_sb = handoff[parity]
                    nc.vector.tensor_mul(oT[po_sl, ct, q_sl], o_raw[0:HD, :],
                                         r_sb)
                return normC

            # ---- schedule -------------------------------------------------
            # DMA order matters at startup: the first QKV matmuls need only
            # xT(group 0) + wq, so those descriptors must head the queues.
            # qkv_gen's first chunk (before its first yield) is the xT DMA;
            # the generator body only touches w*_sb tiles after later yields,
            # by which time the load_w calls below have run.
            qkv0 = qkv_gen(0)
            next(qkv0)                       # emits xT(0) DMA first

            wq_sb = load_w(wq_d, [128, 8, LCOL], "wq_sb")
            tri_sb = consts.tile([128, 2, 128], bf16)
            nc.sync.dma_start(out=tri_sb, in_=tri_d.ap())
            bq_sb = consts.tile([128, 4], f32)
            nc.sync.dma_start(out=bq_sb,
                              in_=bq_d.ap().rearrange("(c p) -> p c", p=128))
            bk_sb = consts.tile([128, 4], f32)
            nc.sync.dma_start(out=bk_sb,
                              in_=bk_d.ap().rearrange("(c p) -> p c", p=128))
            wk_sb = load_w(wk_d, [128, 8, LCOL], "wk_sb")
            wv_sb = load_w(wv_d, [128, 8, LCOL], "wv_sb")
            wp_sb = load_w(wp_d, [128, 4, D], "wp_sb")

            # Prologue: q/k for pair 0 + v for kbs 0-3, dense.
            for _ in range(6):
                next(qkv0)

            defB, defC, defC_next = [], [], []
            for g in range(NSG):
                fill = []
                if g == 0:
                    fill.append(qkv0)        # remaining q/k pairs 1-3
                if g < NSG - 1:
                    fill.append(qkv_gen(g + 1))
                if g == 2:
                    fill.append(proj_gen(0))
                if g == 3:
                    fill.append(proj_gen(1))
                    fill.append(proj_gen(2))
                # g3 stride underfeeds on purpose: the leftover proj chunks
                # drain at the end, covering the final normalize latency.
                stride = {0: 1, 1: 2, 2: 2, 3: 6}[g]
                state = {"i": 0}

                def tick():
                    state["i"] += 1
                    if state["i"] % stride == 0 and fill:
                        try:
                            next(fill[0])
                        except StopIteration:
                            fill.pop(0)

                queue = []
                for ct in range(4):
                    pair_state = {}
                    handoff = {}
                    for kb in range(4 * g + 4):
                        attn_unit(ct, g, kb, pair_state, queue, tick)
                    queue.append(make_normA(ct, g, pair_state, 0, handoff))
                    queue.append(make_normA(ct, g, pair_state, 1, handoff))
                    queue.extend(defB)       # pair ct-1: reciprocal + spread
                    queue.extend(defC)       # pair ct-2: oT multiply
                    defC = defC_next
                    defC_next = [make_normC(ct, g, handoff, 0),
                                 make_normC(ct, g, handoff, 1)]
                    defB = [make_normB(ct, g, handoff, 0),
                            make_normB(ct, g, handoff, 1)]
                while queue:     # group boundary: drain exp/AV + norm chains
                    queue.pop(0)()
                for gen in fill:  # drain any remaining fill chunks
                    for _ in gen:
                        pass
            for fn in defC + defB + defC_next:   # last pairs' norm stages
                fn()
            for _ in proj_gen(NSG - 1):
                pass

            if debug_dump:
                for nm, t in (("qT", qT), ("kT", kT), ("v_ext", v_ext),
                              ("oT", oT)):
                    dmp = nc.dram_tensor(f"dump_{nm}", list(t.shape), bf16,
                                         kind="ExternalOutput")
                    nc.sync.dma_start(out=dmp.ap(), in_=t)

    nc.compile()
    return nc


def _get_nc():
    if "nc" not in _CACHE:
        _CACHE["nc"] = _build()
    return _CACHE["nc"]


def _make_tri():
    """tri[kl, :, c] = 1.0 iff kl <= c (bf16), for 128-wide diagonal blocks,
    duplicated on axis 1 so one DVE multiply covers both heads of a pair."""
    import ml_dtypes
    kl = np.arange(128)[:, None]
    c = np.arange(128)[None, :]
    t = (kl <= c).astype(ml_dtypes.bfloat16)
    return np.ascontiguousarray(np.broadcast_to(t[:, None, :], (128, 2, 128)))


def make_in_maps(x, Wq, bq, Wk, bk, Wv, Wp):
    import ml_dtypes
    bf = ml_dtypes.bfloat16
    tri = _make_tri()
    xt = {}
    wmaps = {}

    def tile_w(w2d, chunks):
        # [128*chunks, n] -> [128, chunks, n] with 128c+p row mapping
        n = w2d.shape[1]
        return np.ascontiguousarray(
            w2d.reshape(chunks, 128, n).transpose(1, 0, 2).astype(bf))

    for hg in range(2):
        hs = slice(hg * HPC, (hg + 1) * HPC)
        wmaps[hg] = {
            "wq": tile_w(Wq[hs].transpose(1, 0, 2).reshape(D, LCOL), 8),
            "wk": tile_w(Wk[hs].transpose(1, 0, 2).reshape(D, LCOL), 8),
            "wv": tile_w(Wv[hs].transpose(1, 0, 2).reshape(D, LCOL), 8),
            "wp": tile_w(Wp[hg * LCOL:(hg + 1) * LCOL, :], 4),
            "bq": np.ascontiguousarray(bq[hs].reshape(LCOL)).astype(np.float32),
            "bk": np.ascontiguousarray(bk[hs].reshape(LCOL)).astype(np.float32),
        }
    in_maps = []
    for c in range(8):
        b, hg = c // 2, c % 2
        if b not in xt:
            # xtg[g, p, c, s] = x[b][512g+s, 128c+p]
            xt[b] = np.ascontiguousarray(
                np.asarray(x[b]).reshape(NSG, SG, 8, 128)
                .transpose(0, 3, 2, 1).astype(bf))
        in_maps.append({"xtg": xt[b], "tri": tri, **wmaps[hg]})
    return in_maps


def combine(results, Wp, bv, bp):
    """Unshard: sum the two head-group partials per batch + linear bias terms."""
    add = bp + bv.reshape(D) @ Wp
    out = np.empty((B, S, D), np.float32)
    for b in range(B):
        out[b] = results[2 * b]["out"] + results[2 * b + 1]["out"] + add
    return out


def kernel(x, Wq, bq, Wk, bk, Wv, bv, Wp, bp):
    from concourse.bass_utils import run_bass_kernel_spmd

    x = np.asarray(x, np.float32)
    Wq = np.asarray(Wq, np.float32)
    Wk = np.asarray(Wk, np.float32)
    Wv = np.asarray(Wv, np.float32)
    bq = np.asarray(bq, np.float32)
    bk = np.asarray(bk, np.float32)
    bv = np.asarray(bv, np.float32)
    Wp = np.asarray(Wp, np.float32)
    bp = np.asarray(bp, np.float32)

    nc = _get_nc()
    in_maps = make_in_maps(x, Wq, bq, Wk, bk, Wv, Wp)
    res = run_bass_kernel_spmd(nc, in_maps, core_ids=list(range(8)))
    return combine(res.results, Wp, bv, bp)
